# revision 10
# baseline (speedup 1.0000x reference)
"""Trainium2 Bass kernel for AttnDecoderRNN single step (batch=1).

8-way tensor parallel, ONE on-device collective (AllGather, 1KB/rank):
  - attention fully replicated per core (attn_W is small): logits + softmax local
  - context computed in full on every core (encoder_outputs replicated),
    weights-stationary so it lands directly in column layout
  - combine (2048x4096) ROW-sharded: each core computes its exact 256-slot
    slice of g = relu(comb_W @ [x;ctx] + b) locally -> AllGather of g (fp32)
  - GRU (2x 6144x2048) row-sharded over output slots -> local gates
  - final gather of h_new slices done on host; attn weights from core 0

Matmul operands are fp16 (single-pass on the PE, half the HBM traffic);
accumulation is fp32 in PSUM and all vector/scalar math stays fp32.
M=1 GEMV matmuls are packed 4-per-PE-pass with column tiling
(tile_position=(0,32j)), with partial rows summed on the vector engines.
The g vector crossing the AllGather uses an interleaved chunk layout
(chunk k = indices {16p+k}) so the DRAM->SBUF reload stays contiguous;
GRU weight rows are permuted on the host to match.  gh = w_hh @ h is
scheduled inside the collective-wait window to keep the PE warm.
"""

import numpy as np

import concourse.bacc as bacc
import concourse.bass as bass
import concourse.mybir as mybir
import concourse.tile as tile
from concourse import bass_utils

F32 = mybir.dt.float32
F16 = mybir.dt.float16
NP16 = np.float16
NCORES = 8
H = 2048          # hidden size
L = 350           # max_length
LP = 384          # padded max_length (3 x 128)
HC = H // 128     # 16 column chunks of h / g
ZC = 2 * HC       # 32 column chunks of [x; h]
GS = 3 * (H // NCORES)   # 768 GRU rows per core (r,z,n x 256)
CS = H // NCORES  # 256: per-core slice of x / ctx / h_new

NEG = -1.0e30

_CACHE = {}


def _build():
    nc = bacc.Bacc(
        "TRN2",
        target_bir_lowering=False,
        debug=False,
        enable_asserts=True,
        num_devices=NCORES,
    )
    rg = [list(range(NCORES))]

    # ---- external inputs (per-core data prepared on host) ----
    d_attn_wt = nc.dram_tensor("attn_wt", [128, ZC * LP], F16, kind="ExternalInput")
    d_attn_b = nc.dram_tensor("attn_b", [1, LP], F32, kind="ExternalInput")
    d_z_cols = nc.dram_tensor("z_cols", [128, ZC], F16, kind="ExternalInput")
    d_enc = nc.dram_tensor("enc_full", [LP, H], F16, kind="ExternalInput")
    d_comb_wt = nc.dram_tensor("comb_wt", [128, ZC * CS], F16, kind="ExternalInput")
    d_comb_b = nc.dram_tensor("comb_b", [1, CS], F32, kind="ExternalInput")
    d_wih = nc.dram_tensor("wih_t", [H, GS], F16, kind="ExternalInput")
    d_whh = nc.dram_tensor("whh_t", [H, GS], F16, kind="ExternalInput")
    d_h_cols = nc.dram_tensor("h_cols", [128, HC], F16, kind="ExternalInput")
    d_hsl = nc.dram_tensor("hsl", [1, CS], F32, kind="ExternalInput")
    d_bih_n = nc.dram_tensor("bih_n", [1, CS], F32, kind="ExternalInput")
    d_bhh_n = nc.dram_tensor("bhh_n", [1, CS], F32, kind="ExternalInput")
    d_bsum_rz = nc.dram_tensor("bsum_rz", [1, 512], F32, kind="ExternalInput")

    # ---- external outputs ----
    d_h_part = nc.dram_tensor("h_part", [1, CS], F32, kind="ExternalOutput")
    d_aw = nc.dram_tensor("aw_full", [1, LP], F32, kind="ExternalOutput")

    ACT = mybir.ActivationFunctionType

    with tile.TileContext(nc) as tc:
        with (
            tc.tile_pool(name="wts", bufs=1) as wp,
            tc.tile_pool(name="work", bufs=1) as wk,
            tc.tile_pool(name="psum", bufs=1, space="PSUM") as ps,
            tc.tile_pool(name="dram", bufs=1, space="DRAM") as dram,
        ):
            # ---------- weight / input DMAs (issue order sets priority) ----------
            z_cols = wp.tile([128, ZC], F16)
            nc.sync.dma_start(z_cols[:], d_z_cols[:])
            attn_b = wp.tile([1, LP], F32)
            nc.sync.dma_start(attn_b[:], d_attn_b[:])
            # attention weights: one tile per col-tiling round (4 k-chunks each)
            attn_w = []
            for i in range(8):
                t = wp.tile([128, 4 * LP], F16, name=f"attn_w{i}")
                nc.sync.dma_start(t[:], d_attn_wt[:, i * 4 * LP:(i + 1) * 4 * LP])
                attn_w.append(t)
            enc_sb = []
            for k in range(3):
                t = wp.tile([128, H], F16, name=f"enc_{k}")
                nc.sync.dma_start(t[:, :H // 2], d_enc[k * 128:(k + 1) * 128, :H // 2])
                nc.sync.dma_start(t[:, H // 2:], d_enc[k * 128:(k + 1) * 128, H // 2:])
                enc_sb.append(t)
            # combine weights: one tile per col-tiling round (4 k-chunks each)
            comb_sb = []
            for i in range(8):
                t = wp.tile([128, 4 * CS], F16, name=f"comb_{i}")
                nc.sync.dma_start(t[:], d_comb_wt[:, i * 4 * CS:(i + 1) * 4 * CS])
                comb_sb.append(t)
            h_cols = wp.tile([128, HC], F16)
            nc.sync.dma_start(h_cols[:], d_h_cols[:])
            hsl = wp.tile([1, CS], F32)
            nc.sync.dma_start(hsl[:], d_hsl[:])
            whh_sb = []
            for k in range(HC):
                t = wp.tile([128, GS], F16, name=f"whh_{k}")
                nc.sync.dma_start(t[:], d_whh[k * 128:(k + 1) * 128, :])
                whh_sb.append(t)
            wih_sb = []
            for k in range(HC):
                t = wp.tile([128, GS], F16, name=f"wih_{k}")
                nc.sync.dma_start(t[:], d_wih[k * 128:(k + 1) * 128, :])
                wih_sb.append(t)
            comb_b = wp.tile([1, CS], F32)
            nc.sync.dma_start(comb_b[:], d_comb_b[:])
            bih_n = wp.tile([1, CS], F32)
            nc.sync.dma_start(bih_n[:], d_bih_n[:])
            bhh_n = wp.tile([1, CS], F32)
            nc.sync.dma_start(bhh_n[:], d_bhh_n[:])
            bsum_rz = wp.tile([1, 512], F32)
            nc.sync.dma_start(bsum_rz[:], d_bsum_rz[:])
            one1 = wp.tile([1, 1], F16)
            nc.vector.memset(one1[:], 1.0)

            # ---------- attention logits (replicated): 4-way col-tiled ----------
            at_ps = ps.tile([128, LP], F32, tag="sp", bufs=1)
            for r in range(8):
                for j in range(4):
                    nc.tensor.matmul(
                        at_ps[32 * j:32 * j + 1, :], z_cols[:, 4 * r + j:4 * r + j + 1],
                        attn_w[r][:, j * LP:(j + 1) * LP],
                        start=(r == 0), stop=(r == 7), tile_position=(0, 32 * j),
                    )
            # sum the 4 strip rows (PSUM) into the SBUF bias, one PSUM read per op
            lga = wk.tile([1, LP], F32)
            nc.vector.tensor_add(lga[:], attn_b[:], at_ps[0:1, :])
            lgc = wk.tile([1, LP], F32)
            nc.vector.tensor_add(lgc[:], lga[:], at_ps[32:33, :])
            lgd = wk.tile([1, LP], F32)
            nc.vector.tensor_add(lgd[:], lgc[:], at_ps[64:65, :])
            lgb = wk.tile([1, LP], F32)
            nc.vector.tensor_add(lgb[:], lgd[:], at_ps[96:97, :])
            exp_row = wk.tile([1, LP], F32)
            nc.scalar.activation(exp_row[:], lgb[:], ACT.Exp)

            # softmax normalization (row layout, replicated)
            tot = wk.tile([1, 1], F32)
            nc.vector.reduce_sum(tot[:], exp_row[:], axis=mybir.AxisListType.X)
            rcp = wk.tile([1, 1], F32)
            nc.vector.reciprocal(rcp[:], tot[:])
            aw_row = wk.tile([1, LP], F32)
            nc.vector.tensor_scalar_mul(aw_row[:], exp_row[:], rcp[:])
            nc.sync.dma_start(d_aw[:], aw_row[:])
            aw16 = wk.tile([1, LP], F16)
            nc.scalar.copy(aw16[:], aw_row[:])

            # transpose aw row -> 3 column chunks via K=1 matmuls
            awc_ps = ps.tile([128, 3], F32, tag="sp2", bufs=1)
            for k in range(3):
                nc.tensor.matmul(awc_ps[:, k:k + 1], aw16[0:1, k * 128:(k + 1) * 128],
                                 one1[:], start=True, stop=True)
            aw_cols = wk.tile([128, 3], F16)
            nc.vector.tensor_copy(aw_cols[:], awc_ps[:])

            # ---------- full context as columns [128,16] (weights stationary) ----
            ctx_ps = ps.tile([128, HC], F32, tag="sp2", bufs=1)
            for m in range(HC):
                for k in range(3):
                    nc.tensor.matmul(
                        ctx_ps[:, m:m + 1], enc_sb[k][:, m * 128:(m + 1) * 128],
                        aw_cols[:, k:k + 1], start=(k == 0), stop=(k == 2),
                    )
            ctx = wk.tile([128, HC], F16)
            nc.vector.tensor_copy(ctx[:], ctx_ps[:])

            # ---------- combine row-shard: exact g slice, 4-way col-tiled --------
            cb_ps = ps.tile([128, CS], F32, tag="cp", bufs=1)
            for r in range(8):
                for j in range(4):
                    k = 4 * r + j
                    lhs = z_cols[:, k:k + 1] if k < HC else ctx[:, k - HC:k - HC + 1]
                    nc.tensor.matmul(
                        cb_ps[32 * j:32 * j + 1, :], lhs,
                        comb_sb[r][:, j * CS:(j + 1) * CS],
                        start=(r == 0), stop=(r == 7), tile_position=(0, 32 * j),
                    )
            ga = wk.tile([1, CS], F32)
            nc.vector.tensor_add(ga[:], comb_b[:], cb_ps[0:1, :])
            gc = wk.tile([1, CS], F32)
            nc.vector.tensor_add(gc[:], ga[:], cb_ps[32:33, :])
            gd = wk.tile([1, CS], F32)
            nc.vector.tensor_add(gd[:], gc[:], cb_ps[64:65, :])
            gbias = wk.tile([1, CS], F32)
            nc.vector.tensor_add(gbias[:], gd[:], cb_ps[96:97, :])
            g_slice = wk.tile([1, CS], F32)
            nc.scalar.activation(g_slice[:], gbias[:], ACT.Relu)

            # ---------- AllGather the finished g slices (fp32, 1KB/rank) ---------
            cc_in = dram.tile([1, CS], F32)
            cc_out = dram.tile([1, H], F32, addr_space="Shared")
            nc.sync.dma_start(cc_in[:], g_slice[:])
            nc.gpsimd.collective_compute(
                "AllGather", mybir.AluOpType.bypass, replica_groups=rg,
                ins=[cc_in[:]], outs=[cc_out[:]],
            )

            # ---------- gh row = (w_hh_slice @ h).T in the collective window -----
            gh_ps_a = ps.tile([1, 512], F32, tag="gha")
            gh_ps_b = ps.tile([1, GS - 512], F32, tag="ghb")
            for (t, n0, nw) in ((gh_ps_a, 0, 512), (gh_ps_b, 512, GS - 512)):
                for k in range(HC):
                    nc.tensor.matmul(
                        t[:], h_cols[:, k:k + 1], whh_sb[k][:, n0:n0 + nw],
                        start=(k == 0), stop=(k == HC - 1),
                    )

            # load gathered g as [128,16] (chunk k = indices {16p+k}), cast fp16
            gsum = wk.tile([128, HC], F32)
            nc.sync.dma_start(gsum[:], cc_out[0, :].rearrange("(p k) -> p k", k=HC))
            g = wk.tile([128, HC], F16)
            nc.vector.tensor_copy(g[:], gsum[:])

            # ---------- gi row = (w_ih_slice @ g).T, 4-way col-tiled -------------
            gi_ps_a = ps.tile([128, 512], F32, tag="gia")
            gi_ps_b = ps.tile([128, GS - 512], F32, tag="gib")
            for (t, n0, nw) in ((gi_ps_a, 0, 512), (gi_ps_b, 512, GS - 512)):
                for r in range(4):
                    for j in range(4):
                        k = 4 * r + j
                        nc.tensor.matmul(
                            t[32 * j:32 * j + 1, :], g[:, k:k + 1],
                            wih_sb[k][:, n0:n0 + nw],
                            start=(r == 0), stop=(r == 3), tile_position=(0, 32 * j),
                        )

            # ---------- GRU gates on the local 256-slot slice (row layout) -------
            # r/z part: sum gi strips + gh + combined bias (one PSUM read per op)
            ia1 = wk.tile([1, 512], F32)
            nc.vector.tensor_add(ia1[:], bsum_rz[:], gi_ps_a[0:1, :])
            ia2 = wk.tile([1, 512], F32)
            nc.vector.tensor_add(ia2[:], ia1[:], gi_ps_a[32:33, :])
            ia3 = wk.tile([1, 512], F32)
            nc.vector.tensor_add(ia3[:], ia2[:], gi_ps_a[64:65, :])
            ia4 = wk.tile([1, 512], F32)
            nc.vector.tensor_add(ia4[:], ia3[:], gi_ps_a[96:97, :])
            rzs = wk.tile([1, 512], F32)
            nc.vector.tensor_add(rzs[:], ia4[:], gh_ps_a[:])
            rz = wk.tile([1, 512], F32)
            nc.scalar.activation(rz[:], rzs[:], ACT.Sigmoid)
            # n part
            ib1 = wk.tile([1, CS], F32)
            nc.vector.tensor_add(ib1[:], bih_n[:], gi_ps_b[0:1, :])
            ib2 = wk.tile([1, CS], F32)
            nc.vector.tensor_add(ib2[:], ib1[:], gi_ps_b[32:33, :])
            ib3 = wk.tile([1, CS], F32)
            nc.vector.tensor_add(ib3[:], ib2[:], gi_ps_b[64:65, :])
            i_n = wk.tile([1, CS], F32)
            nc.vector.tensor_add(i_n[:], ib3[:], gi_ps_b[96:97, :])
            h_n = wk.tile([1, CS], F32)
            nc.vector.tensor_add(h_n[:], bhh_n[:], gh_ps_b[:])
            t1 = wk.tile([1, CS], F32)
            nc.vector.tensor_mul(t1[:], rz[:, 0:CS], h_n[:])
            t2 = wk.tile([1, CS], F32)
            nc.vector.tensor_add(t2[:], t1[:], i_n[:])
            nt = wk.tile([1, CS], F32)
            nc.scalar.activation(nt[:], t2[:], ACT.Tanh)
            hmn = wk.tile([1, CS], F32)
            nc.vector.tensor_sub(hmn[:], hsl[:], nt[:])
            zt = wk.tile([1, CS], F32)
            nc.vector.tensor_mul(zt[:], rz[:, CS:512], hmn[:])
            hnew = wk.tile([1, CS], F32)
            nc.vector.tensor_add(hnew[:], nt[:], zt[:])
            nc.sync.dma_start(d_h_part[:], hnew[:])

    nc.compile()
    return nc


def _prep(inputs):
    """Build per-core input maps from the full problem inputs."""
    f = lambda a: np.ascontiguousarray(np.asarray(a, dtype=np.float32))
    x = f(inputs["input"]).reshape(H)
    h = f(inputs["hidden"]).reshape(H)
    enc = f(inputs["encoder_outputs"])
    attn_W = f(inputs["attn_W"])
    attn_b = f(inputs["attn_b"])
    comb_W = f(inputs["comb_W"])
    comb_b = f(inputs["comb_b"])
    w_ih = f(inputs["w_ih"])
    w_hh = f(inputs["w_hh"])
    b_ih = f(inputs["b_ih"])
    b_hh = f(inputs["b_hh"])

    z = np.concatenate([x, h])
    z_cols = np.ascontiguousarray(z.reshape(ZC, 128).T.astype(NP16))
    h_cols = np.ascontiguousarray(h.astype(NP16).reshape(128, HC))   # interleaved

    Wp = np.zeros((LP, 2 * H), np.float32)
    Wp[:L] = attn_W
    bp = np.full((1, LP), NEG, np.float32)
    bp[0, :L] = attn_b
    encp16 = np.zeros((LP, H), NP16)
    encp16[:L] = enc.astype(NP16)
    # replicated attention weights, packed for k-chunked rhs access
    attn_wt = np.ascontiguousarray(
        Wp.T.reshape(ZC, 128, LP).transpose(1, 0, 2).reshape(128, ZC * LP).astype(NP16))
    # row permutation so k-chunk k of the GRU contraction = g indices {16p+k}
    perm = np.add.outer(np.arange(HC), HC * np.arange(128)).reshape(-1)

    in_maps = []
    for c in range(NCORES):
        sel = np.concatenate([np.arange(c * CS, (c + 1) * CS) + g * H for g in range(3)])
        # comb row-slice: (256 out, 4096 in) -> W^T (4096, 256) packed by k-chunk
        cwt = comb_W[c * CS:(c + 1) * CS].T.astype(NP16)  # (4096, 256)
        comb_wt = np.ascontiguousarray(
            cwt.reshape(ZC, 128, CS).transpose(1, 0, 2).reshape(128, ZC * CS))
        bsum = (b_ih[sel] + b_hh[sel])[:512].reshape(1, 512)
        in_maps.append({
            "attn_wt": attn_wt,
            "attn_b": bp,
            "z_cols": z_cols,
            "enc_full": encp16,
            "comb_wt": comb_wt,
            "comb_b": np.ascontiguousarray(comb_b[c * CS:(c + 1) * CS].reshape(1, CS)),
            "wih_t": np.ascontiguousarray(w_ih[sel].T[perm].astype(NP16)),
            "whh_t": np.ascontiguousarray(w_hh[sel].T[perm].astype(NP16)),
            "h_cols": h_cols,
            "hsl": np.ascontiguousarray(h[c * CS:(c + 1) * CS].reshape(1, CS)),
            "bih_n": np.ascontiguousarray(b_ih[sel][512:].reshape(1, CS)),
            "bhh_n": np.ascontiguousarray(b_hh[sel][512:].reshape(1, CS)),
            "bsum_rz": np.ascontiguousarray(bsum),
        })
    return in_maps


def kernel(**inputs):
    if "nc" not in _CACHE:
        _CACHE["nc"] = _build()
    nc = _CACHE["nc"]
    in_maps = _prep(inputs)
    res = bass_utils.run_bass_kernel_spmd(
        nc, in_maps, core_ids=list(range(NCORES)), **_CACHE.get("run_kwargs", {}))
    _CACHE["last_result"] = res

    h_full = np.concatenate(
        [np.asarray(res.results[c]["h_part"]).reshape(CS) for c in range(NCORES)])
    aw_full = np.asarray(res.results[0]["aw_full"]).reshape(LP)[:L]
    out = h_full.reshape(1, 1, H).astype(np.float32)
    return (out, out.copy(), aw_full.reshape(1, L).astype(np.float32))
